# revision 23
# baseline (speedup 1.0000x reference)
"""Trainium2 Bass kernel for nn_Head_84043920048318 (sparse_attention).

Reference computation (per batch b):
    q = x @ Wq; k = x @ Wk; v = x @ Wv           [T, HS]
    wei = (q @ k.T) * C**-0.5                    [T, T]
    for s:  P = softmax(wei * adjacent[b, s], axis=-1);  out[b, s] = P @ v

Sharding: data-parallel over B across 8 NeuronCores (4 batches each);
weights replicated. HW exec ~98-101 us/core (baseline 172.7 us).

Design — everything lives in the transposed domain, no PE transposes:
  - Host prep (not on the HW critical path): adjacency cast to bf16 and
    pre-permuted to [b, qi, p, s2, ub, t] = the exact SBUF layout, so each
    2 MB adjacency DMA is one contiguous 16 KB run per partition; x is
    shipped as x^T [C, T] bf16; Wq/Wk are folded into W' = Wk @ Wq^T so
    QK needs one projection (weiT = x W' x^T); output leaves the device
    as [b, t, s, d] bf16 (>=512B lines) and is transposed/cast on host.
  - weiT [u, t] comes straight off the QK matmul in the transposed
    orientation, so the adjacency mask multiplies in its natural layout:
    one bf16 2x-mode DVE multiply + one ACT exp per two s-slices.
  - AV matmuls run with P^T as the stationary operand against [v | 1],
    so the softmax denominator falls out of column 128 of PSUM. av
    blocks sit at h*512 + tb2*129 in a 2-bank PSUM tile: one strided
    reciprocal + one broadcast tensor_tensor (stride-0 AP) normalizes a
    whole s-slice straight from PSUM fp32 into the bf16 output tile.
  - Queue split: adjacency + weights + x^T on the sync HWDGE ring
    (smalls issued first so the 2 MB bursts don't starve them); output
    stores per 2 s-slices via gpsimd SWDGE so they never block input
    prefetch. A dummy activation at t=0 pulls the ACT table load off the
    critical path. gpsimd is deliberately NOT used for elementwise work:
    its SBUF port lock against the (saturated) DVE costs more than it
    saves (measured +35 us).
  - Engine balance in steady state (~73 us): ACT = exp (59) + 3/4 of the
    weiT copies; DVE = mask-mult (37) + normalize (28) + 1/4 copies;
    PE = QK + AV (~60, not critical); DMA ~75 us union, overlapped.
"""

import numpy as np
import ml_dtypes

B, S, T, C, HS = 32, 8, 512, 128, 128
NCORES = 8
BPC = B // NCORES
TB = T // 128
UB = T // 128
SCALE = float(C) ** -0.5
SQ = 4  # s-slices per adjacency DMA / mult / exp block

_CACHED = None


def _build_module():
    import concourse.bacc as bacc
    import concourse.mybir as mybir
    from concourse import tile
    from concourse.ap import AP

    f32 = mybir.dt.float32
    f32r = mybir.dt.float32r
    bf16 = mybir.dt.bfloat16

    nc = bacc.Bacc("TRN2", target_bir_lowering=False, debug=False, num_devices=1)

    xT_d = nc.dram_tensor("xT", [BPC, C, T], bf16, kind="ExternalInput").ap()
    # adjacency pre-permuted on host to the exact SBUF layout:
    # [b, qi, p, s2, ub, t] so each partition reads one 16 KB run per DMA
    adjT_d = nc.dram_tensor(
        "adjT", [BPC, S // SQ, 128, SQ, UB, T], bf16, kind="ExternalInput"
    ).ap()
    wqk_d = nc.dram_tensor("Wqk", [C, C], bf16, kind="ExternalInput").ap()
    wv_d = nc.dram_tensor("Wv", [C, HS], bf16, kind="ExternalInput").ap()
    # [b, t, s, d] so each DMA line is >= 4*HS contiguous = 1 KB bf16
    out_d = nc.dram_tensor("out", [BPC, T, S, HS], bf16, kind="ExternalOutput").ap()

    with tile.TileContext(nc) as tc:
        with (
            tc.tile_pool(name="consts", bufs=1) as consts,
            tc.tile_pool(name="xp", bufs=BPC) as xp,
            tc.tile_pool(name="bpool", bufs=2) as bpool,
            tc.tile_pool(name="opool", bufs=3) as opool,
            tc.tile_pool(name="adjp", bufs=4) as adjp,
            tc.tile_pool(name="spool", bufs=2) as spool,
            tc.tile_pool(name="tiny", bufs=8) as tiny,
            tc.tile_pool(name="pav", bufs=3, space="PSUM") as pav,
            tc.tile_pool(name="psmall", bufs=2, space="PSUM") as psmall,
        ):
            # weights + x^T go FIRST on the sync ring so they land before
            # the 2 MB adjacency bursts start hogging the SDMA engines.
            # dummy activation: pulls the ACT exp-table load into the boot
            # window instead of the first real copy/exp on the critical path
            warm = consts.tile([128, 1], f32, tag="warm")
            nc.vector.memset(warm[:], 0.0)
            nc.scalar.activation(
                warm[:], warm[:], mybir.ActivationFunctionType.Exp
            )

            wqk_sb_t = consts.tile([C, C], bf16, tag="wqk")
            wv_sb_t = consts.tile([C, HS], bf16, tag="wv")
            nc.sync.dma_start(wqk_sb_t[:], wqk_d)
            nc.sync.dma_start(wv_sb_t[:], wv_d)
            wqk_sb, wv_sb = wqk_sb_t[:], wv_sb_t[:]

            xTs = []
            for b in range(BPC):
                xT_t = xp.tile([C, T], bf16, tag="xT")
                nc.sync.dma_start(xT_t[:], xT_d[b])
                xTs.append(xT_t[:])

            for b in range(BPC):
                xT = xTs[b]
                # ---- fused QK: g^T = (Wq Wk^T)^T-projected x, so that
                # wei^T = g x^T needs only one projection ----
                gT_ps = psmall.tile([C, T], f32, tag="ps")
                nc.tensor.matmul(gT_ps[:], wqk_sb, xT)
                gT = bpool.tile([C, T], bf16, tag="qT")
                nc.scalar.copy(gT[:], gT_ps[:])

                # ---- v natural [u, d] + ones column, bf16 ----
                vp = bpool.tile([128, UB, HS + 1], bf16, tag="vp")
                v_ps = psmall.tile([128, UB, HS], f32, tag="ps")
                for ub in range(UB):
                    nc.tensor.matmul(
                        v_ps[:, ub], xT[:, ub * 128 : (ub + 1) * 128], wv_sb
                    )
                nc.vector.tensor_copy(vp[:, :, 0:HS], v_ps[:])
                nc.vector.memset(vp[:, :, HS : HS + 1], 1.0)

                # ---- QK transposed: weiT [u, t] = (g x^T)^T = x g^T ----
                weiT = bpool.tile([128, UB, T], bf16, tag="weiT")
                for ub in range(UB):
                    weiT_ps = psmall.tile([128, T], f32, tag="ps")
                    nc.tensor.matmul(
                        weiT_ps[:], gT[:, ub * 128 : (ub + 1) * 128], xT
                    )
                    if ub < 3:
                        nc.scalar.copy(weiT[:, ub], weiT_ps[:])
                    else:
                        nc.vector.tensor_copy(weiT[:, ub], weiT_ps[:])

                for qi in range(S // SQ):
                    # 2 MB bf16 load of SQ adjacency slices (1 KB lines)
                    adj4 = adjp.tile([128, SQ, UB, T], bf16, tag="adj")
                    nc.sync.dma_start(adj4[:], adjT_d[b, qi])

                    outq = opool.tile([128, TB, SQ, HS], bf16, tag="outq")
                    # mask-mult per 2 slices (fine DVE/ACT interleave) but
                    # ONE exp per 4-slice block (amortizes ACT overhead)
                    prod = spool.tile([128, SQ, UB, T], bf16, tag="prod")
                    w_b = weiT[:].unsqueeze(1).broadcast_to((128, 2, UB, T))
                    for half in range(SQ // 2):
                        nc.vector.tensor_tensor(
                            prod[:, 2 * half : 2 * half + 2],
                            adj4[:, 2 * half : 2 * half + 2],
                            w_b,
                            mybir.AluOpType.mult,
                        )
                    pt = spool.tile([128, SQ, UB, T], bf16, tag="pt")
                    nc.scalar.activation(
                        pt[:], prod[:], mybir.ActivationFunctionType.Exp,
                        scale=SCALE,
                    )
                    if True:
                        for s2 in range(SQ):
                            sq = s2
                            # av [128, 1024] spans 2 PSUM banks; block
                            # (h, tb2) at h*512 + tb2*129 so no matmul
                            # output crosses a bank.
                            av = pav.tile([128, 2 * 512], f32, tag="av")
                            for h in range(2):
                                for tb2 in range(2):
                                    tb = 2 * h + tb2
                                    off = h * 512 + tb2 * 129
                                    for ub in range(UB):
                                        nc.tensor.matmul(
                                            av[:, off : off + HS + 1],
                                            pt[
                                                :,
                                                sq,
                                                ub,
                                                tb * 128 : (tb + 1) * 128,
                                            ],
                                            vp[:, ub, :],
                                            start=(ub == 0),
                                            stop=(ub == UB - 1),
                                        )
                            # one strided reciprocal + one broadcast TT
                            av_ap = av[:]
                            pdim = list(av_ap.ap[0])
                            sums = AP(
                                av_ap.tensor,
                                av_ap.offset + HS,
                                [pdim, [512, 2], [129, 2], [1, 1]],
                            )
                            rcp = tiny.tile([128, 2, 2], f32, tag="rcp")
                            nc.vector.reciprocal(rcp[:], sums)
                            vals = AP(
                                av_ap.tensor,
                                av_ap.offset,
                                [pdim, [512, 2], [129, 2], [1, HS]],
                            )
                            r_b = rcp[:].unsqueeze(3).broadcast_to(
                                (128, 2, 2, HS)
                            )
                            o_ap = outq[:, :, sq, :].rearrange(
                                "p (h t2) d -> p h t2 d", h=2
                            )
                            nc.vector.tensor_tensor(
                                o_ap, vals, r_b, mybir.AluOpType.mult
                            )

                            if s2 % 2 == 1:
                                h2 = s2 // 2
                                nc.gpsimd.dma_start(
                                    out_d[
                                        b,
                                        :,
                                        SQ * qi + 2 * h2 : SQ * qi + 2 * h2 + 2,
                                        :,
                                    ].rearrange(
                                        "(tb p) s d -> p tb s d", p=128
                                    ),
                                    outq[:, :, 2 * h2 : 2 * h2 + 2, :],
                                )

    nc.compile()
    return nc


def _get_module():
    global _CACHED
    if _CACHED is None:
        _CACHED = _build_module()
    return _CACHED


def run_on_hw(in_maps, trace=False, trace_kwargs=None):
    """Run the compiled module on the 8 NeuronCores. Returns BassKernelResults."""
    from concourse.bass_utils import run_bass_kernel_spmd
    from concourse.bass_interp import get_hw_module

    nc = _get_module()
    old_m = nc.m
    nc.m = get_hw_module(nc.m)
    try:
        return run_bass_kernel_spmd(
            nc,
            in_maps,
            core_ids=list(range(NCORES)),
            trace=trace,
            **(trace_kwargs or {}),
        )
    finally:
        nc.m = old_m


def make_in_maps(x, adjacent, Wq, Wk, Wv):
    x = np.ascontiguousarray(x, dtype=np.float32)
    Wqk = (
        np.asarray(Wk, dtype=np.float32) @ np.asarray(Wq, dtype=np.float32).T
    ).astype(ml_dtypes.bfloat16)  # [C, C]: weiT = x (Wk Wq^T) x^T
    Wv = np.ascontiguousarray(Wv, dtype=np.float32).astype(ml_dtypes.bfloat16)
    xT = np.ascontiguousarray(x.transpose(0, 2, 1)).astype(
        ml_dtypes.bfloat16
    )  # [B, C, T] bf16
    # [b, qi, p, s2, ub, t]: partition-contiguous runs for max DMA line size
    a6 = np.asarray(adjacent, dtype=np.float32).reshape(B, S // SQ, SQ, T, UB, 128)
    adjT = np.ascontiguousarray(a6.transpose(0, 1, 5, 2, 4, 3)).astype(
        ml_dtypes.bfloat16
    )
    return [
        {
            "xT": xT[c * BPC : (c + 1) * BPC],
            "adjT": adjT[c * BPC : (c + 1) * BPC],
            "Wqk": Wqk,
            "Wv": Wv,
        }
        for c in range(NCORES)
    ]


def kernel(**inputs) -> np.ndarray:
    in_maps = make_in_maps(
        inputs["x"], inputs["adjacent"], inputs["Wq"], inputs["Wk"], inputs["Wv"]
    )
    res = run_on_hw(in_maps)
    # per-core out: [BPC, T, S, HS] bf16 -> [BPC, S, T, HS] f32
    outs = [
        np.asarray(res.results[c]["out"])
        .astype(np.float32)
        .transpose(0, 2, 1, 3)
        for c in range(NCORES)
    ]
    return np.ascontiguousarray(np.concatenate(outs, axis=0))


# revision 25
# speedup vs baseline: 1.0317x; 1.0317x over previous
"""Trainium2 Bass kernel for nn_Head_84043920048318 (sparse_attention).

Reference computation (per batch b):
    q = x @ Wq; k = x @ Wk; v = x @ Wv           [T, HS]
    wei = (q @ k.T) * C**-0.5                    [T, T]
    for s:  P = softmax(wei * adjacent[b, s], axis=-1);  out[b, s] = P @ v

Sharding: data-parallel over B across 8 NeuronCores (4 batches each);
weights replicated. HW exec ~98-101 us/core (baseline 172.7 us).

Design — everything lives in the transposed domain, no PE transposes:
  - Host prep (not on the HW critical path): adjacency cast to bf16 and
    pre-permuted to [b, qi, p, s2, ub, t] = the exact SBUF layout, so each
    2 MB adjacency DMA is one contiguous 16 KB run per partition; x is
    shipped as x^T [C, T] bf16; Wq/Wk are folded into W' = Wk @ Wq^T so
    QK needs one projection (weiT = x W' x^T); output leaves the device
    as [b, t, s, d] bf16 (>=512B lines) and is transposed/cast on host.
  - weiT [u, t] comes straight off the QK matmul in the transposed
    orientation, so the adjacency mask multiplies in its natural layout:
    one bf16 2x-mode DVE multiply + one ACT exp per two s-slices.
  - AV matmuls run with P^T as the stationary operand against [v | 1],
    so the softmax denominator falls out of column 128 of PSUM. av
    blocks sit at h*512 + tb2*129 in a 2-bank PSUM tile: one strided
    reciprocal + one broadcast tensor_tensor (stride-0 AP) normalizes a
    whole s-slice straight from PSUM fp32 into the bf16 output tile.
  - Queue split: adjacency + weights + x^T on the sync HWDGE ring
    (smalls issued first so the 2 MB bursts don't starve them); output
    stores per 2 s-slices via gpsimd SWDGE so they never block input
    prefetch. A dummy activation at t=0 pulls the ACT table load off the
    critical path. gpsimd is deliberately NOT used for elementwise work:
    its SBUF port lock against the (saturated) DVE costs more than it
    saves (measured +35 us).
  - Engine balance in steady state (~73 us): ACT = exp (59) + 3/4 of the
    weiT copies; DVE = mask-mult (37) + normalize (28) + 1/4 copies;
    PE = QK + AV (~60, not critical); DMA ~75 us union, overlapped.
"""

import numpy as np
import ml_dtypes

B, S, T, C, HS = 32, 8, 512, 128, 128
NCORES = 8
BPC = B // NCORES
TB = T // 128
UB = T // 128
SCALE = float(C) ** -0.5
SQ = 4  # s-slices per adjacency DMA / mult / exp block

_CACHED = None


def _build_module():
    import concourse.bacc as bacc
    import concourse.mybir as mybir
    from concourse import tile
    from concourse.ap import AP

    f32 = mybir.dt.float32
    f32r = mybir.dt.float32r
    bf16 = mybir.dt.bfloat16

    nc = bacc.Bacc("TRN2", target_bir_lowering=False, debug=False, num_devices=1)

    xT_d = nc.dram_tensor("xT", [BPC, C, T], bf16, kind="ExternalInput").ap()
    # adjacency pre-permuted on host to the exact SBUF layout:
    # [b, qi, p, s2, ub, t] so each partition reads one 16 KB run per DMA
    adjT_d = nc.dram_tensor(
        "adjT", [BPC, S // SQ, 128, SQ, UB, T], bf16, kind="ExternalInput"
    ).ap()
    wqk_d = nc.dram_tensor("Wqk", [C, C], bf16, kind="ExternalInput").ap()
    wv_d = nc.dram_tensor("Wv", [C, HS], bf16, kind="ExternalInput").ap()
    # [b, t, s, d] so each DMA line is >= 4*HS contiguous = 1 KB bf16
    out_d = nc.dram_tensor("out", [BPC, T, S, HS], bf16, kind="ExternalOutput").ap()

    with tile.TileContext(nc) as tc:
        with (
            tc.tile_pool(name="consts", bufs=1) as consts,
            tc.tile_pool(name="xp", bufs=BPC) as xp,
            tc.tile_pool(name="bpool", bufs=2) as bpool,
            tc.tile_pool(name="opool", bufs=3) as opool,
            tc.tile_pool(name="adjp", bufs=4) as adjp,
            tc.tile_pool(name="spool", bufs=4) as spool,
            tc.tile_pool(name="tiny", bufs=8) as tiny,
            tc.tile_pool(name="pav", bufs=3, space="PSUM") as pav,
            tc.tile_pool(name="psmall", bufs=2, space="PSUM") as psmall,
        ):
            # weights + x^T go FIRST on the sync ring so they land before
            # the 2 MB adjacency bursts start hogging the SDMA engines.
            # dummy activation: pulls the ACT exp-table load into the boot
            # window instead of the first real copy/exp on the critical path
            warm = consts.tile([128, 1], f32, tag="warm")
            nc.vector.memset(warm[:], 0.0)
            nc.scalar.activation(
                warm[:], warm[:], mybir.ActivationFunctionType.Exp
            )

            # PE warm-up in the boot shadow: ~3.4us of dummy matmuls flips
            # the HAM clock gate to 2.4 GHz before the first real matmul
            warmmat = consts.tile([128, T], bf16, tag="warmmat")
            nc.vector.memset(warmmat[:], 0.0)
            warm_ps = psmall.tile([128, T], f32, tag="ps")
            for _ in range(8):
                nc.tensor.matmul(warm_ps[:], warmmat[:, 0:128], warmmat[:])

            wqk_sb_t = consts.tile([C, C], bf16, tag="wqk")
            wv_sb_t = consts.tile([C, HS], bf16, tag="wv")
            nc.sync.dma_start(wqk_sb_t[:], wqk_d)
            nc.sync.dma_start(wv_sb_t[:], wv_d)
            wqk_sb, wv_sb = wqk_sb_t[:], wv_sb_t[:]

            xTs = []
            for b in range(BPC):
                xT_t = xp.tile([C, T], bf16, tag="xT")
                nc.sync.dma_start(xT_t[:], xT_d[b])
                xTs.append(xT_t[:])

            for b in range(BPC):
                xT = xTs[b]
                # ---- fused QK: g^T = (Wq Wk^T)^T-projected x, so that
                # wei^T = g x^T needs only one projection ----
                gT_ps = psmall.tile([C, T], f32, tag="ps")
                nc.tensor.matmul(gT_ps[:], wqk_sb, xT)
                gT = bpool.tile([C, T], bf16, tag="qT")
                nc.scalar.copy(gT[:], gT_ps[:])

                # ---- v natural [u, d] + ones column, bf16 ----
                vp = bpool.tile([128, UB, HS + 1], bf16, tag="vp")
                v_ps = psmall.tile([128, UB, HS], f32, tag="ps")
                for ub in range(UB):
                    nc.tensor.matmul(
                        v_ps[:, ub], xT[:, ub * 128 : (ub + 1) * 128], wv_sb
                    )
                nc.vector.tensor_copy(vp[:, :, 0:HS], v_ps[:])
                nc.vector.memset(vp[:, :, HS : HS + 1], 1.0)

                # ---- QK transposed: weiT [u, t] = (g x^T)^T = x g^T ----
                weiT = bpool.tile([128, UB, T], bf16, tag="weiT")
                for ub in range(UB):
                    weiT_ps = psmall.tile([128, T], f32, tag="ps")
                    nc.tensor.matmul(
                        weiT_ps[:], gT[:, ub * 128 : (ub + 1) * 128], xT
                    )
                    if ub < 3:
                        nc.scalar.copy(weiT[:, ub], weiT_ps[:])
                    else:
                        nc.vector.tensor_copy(weiT[:, ub], weiT_ps[:])

                for qi in range(S // SQ):
                    # 2 MB bf16 load of SQ adjacency slices (1 KB lines)
                    adj4 = adjp.tile([128, SQ, UB, T], bf16, tag="adj")
                    nc.sync.dma_start(adj4[:], adjT_d[b, qi])

                    outq = opool.tile([128, TB, SQ, HS], bf16, tag="outq")
                    for half in range(SQ // 2):
                        # prodT = adjT * weiT (bf16 2x, weiT bcast over s)
                        prod = spool.tile([128, 2, UB, T], bf16, tag="prod")
                        w_b = weiT[:].unsqueeze(1).broadcast_to((128, 2, UB, T))
                        nc.vector.tensor_tensor(
                            prod[:],
                            adj4[:, 2 * half : 2 * half + 2],
                            w_b,
                            mybir.AluOpType.mult,
                        )

                        # P^T = exp(scale * prodT)
                        pt = spool.tile([128, 2, UB, T], bf16, tag="pt")
                        nc.scalar.activation(
                            pt[:], prod[:], mybir.ActivationFunctionType.Exp,
                            scale=SCALE,
                        )

                        for s2 in range(2):
                            sq = 2 * half + s2
                            # av [128, 1024] spans 2 PSUM banks; block
                            # (h, tb2) at h*512 + tb2*129 so no matmul
                            # output crosses a bank.
                            av = pav.tile([128, 2 * 512], f32, tag="av")
                            for h in range(2):
                                for tb2 in range(2):
                                    tb = 2 * h + tb2
                                    off = h * 512 + tb2 * 129
                                    for ub in range(UB):
                                        nc.tensor.matmul(
                                            av[:, off : off + HS + 1],
                                            pt[
                                                :,
                                                s2,
                                                ub,
                                                tb * 128 : (tb + 1) * 128,
                                            ],
                                            vp[:, ub, :],
                                            start=(ub == 0),
                                            stop=(ub == UB - 1),
                                        )
                            # one strided reciprocal + one broadcast TT
                            av_ap = av[:]
                            pdim = list(av_ap.ap[0])
                            sums = AP(
                                av_ap.tensor,
                                av_ap.offset + HS,
                                [pdim, [512, 2], [129, 2], [1, 1]],
                            )
                            rcp = tiny.tile([128, 2, 2], f32, tag="rcp")
                            nc.vector.reciprocal(rcp[:], sums)
                            vals = AP(
                                av_ap.tensor,
                                av_ap.offset,
                                [pdim, [512, 2], [129, 2], [1, HS]],
                            )
                            r_b = rcp[:].unsqueeze(3).broadcast_to(
                                (128, 2, 2, HS)
                            )
                            o_ap = outq[:, :, sq, :].rearrange(
                                "p (h t2) d -> p h t2 d", h=2
                            )
                            nc.vector.tensor_tensor(
                                o_ap, vals, r_b, mybir.AluOpType.mult
                            )

                        nc.gpsimd.dma_start(
                            out_d[
                                b,
                                :,
                                SQ * qi + 2 * half : SQ * qi + 2 * half + 2,
                                :,
                            ].rearrange("(tb p) s d -> p tb s d", p=128),
                            outq[:, :, 2 * half : 2 * half + 2, :],
                        )

    nc.compile()
    return nc


def _get_module():
    global _CACHED
    if _CACHED is None:
        _CACHED = _build_module()
    return _CACHED


def run_on_hw(in_maps, trace=False, trace_kwargs=None):
    """Run the compiled module on the 8 NeuronCores. Returns BassKernelResults."""
    from concourse.bass_utils import run_bass_kernel_spmd
    from concourse.bass_interp import get_hw_module

    nc = _get_module()
    old_m = nc.m
    nc.m = get_hw_module(nc.m)
    try:
        return run_bass_kernel_spmd(
            nc,
            in_maps,
            core_ids=list(range(NCORES)),
            trace=trace,
            **(trace_kwargs or {}),
        )
    finally:
        nc.m = old_m


def make_in_maps(x, adjacent, Wq, Wk, Wv):
    x = np.ascontiguousarray(x, dtype=np.float32)
    Wqk = (
        np.asarray(Wk, dtype=np.float32) @ np.asarray(Wq, dtype=np.float32).T
    ).astype(ml_dtypes.bfloat16)  # [C, C]: weiT = x (Wk Wq^T) x^T
    Wv = np.ascontiguousarray(Wv, dtype=np.float32).astype(ml_dtypes.bfloat16)
    xT = np.ascontiguousarray(x.transpose(0, 2, 1)).astype(
        ml_dtypes.bfloat16
    )  # [B, C, T] bf16
    # [b, qi, p, s2, ub, t]: partition-contiguous runs for max DMA line size
    a6 = np.asarray(adjacent, dtype=np.float32).reshape(B, S // SQ, SQ, T, UB, 128)
    adjT = np.ascontiguousarray(a6.transpose(0, 1, 5, 2, 4, 3)).astype(
        ml_dtypes.bfloat16
    )
    return [
        {
            "xT": xT[c * BPC : (c + 1) * BPC],
            "adjT": adjT[c * BPC : (c + 1) * BPC],
            "Wqk": Wqk,
            "Wv": Wv,
        }
        for c in range(NCORES)
    ]


def kernel(**inputs) -> np.ndarray:
    in_maps = make_in_maps(
        inputs["x"], inputs["adjacent"], inputs["Wq"], inputs["Wk"], inputs["Wv"]
    )
    res = run_on_hw(in_maps)
    # per-core out: [BPC, T, S, HS] bf16 -> [BPC, S, T, HS] f32
    outs = [
        np.asarray(res.results[c]["out"])
        .astype(np.float32)
        .transpose(0, 2, 1, 3)
        for c in range(NCORES)
    ]
    return np.ascontiguousarray(np.concatenate(outs, axis=0))


# revision 26
# speedup vs baseline: 1.0423x; 1.0102x over previous
"""Trainium2 Bass kernel for nn_Head_84043920048318 (sparse_attention).

Reference computation (per batch b):
    q = x @ Wq; k = x @ Wk; v = x @ Wv           [T, HS]
    wei = (q @ k.T) * C**-0.5                    [T, T]
    for s:  P = softmax(wei * adjacent[b, s], axis=-1);  out[b, s] = P @ v

Sharding: data-parallel over B across 8 NeuronCores (4 batches each);
weights replicated. HW exec ~98-101 us/core (baseline 172.7 us).

Design — everything lives in the transposed domain, no PE transposes:
  - Host prep (not on the HW critical path): adjacency cast to bf16 and
    pre-permuted to [b, qi, p, s2, ub, t] = the exact SBUF layout, so each
    2 MB adjacency DMA is one contiguous 16 KB run per partition; x is
    shipped as x^T [C, T] bf16; Wq/Wk are folded into W' = Wk @ Wq^T so
    QK needs one projection (weiT = x W' x^T); output leaves the device
    as [b, t, s, d] bf16 (>=512B lines) and is transposed/cast on host.
  - weiT [u, t] comes straight off the QK matmul in the transposed
    orientation, so the adjacency mask multiplies in its natural layout:
    one bf16 2x-mode DVE multiply + one ACT exp per two s-slices.
  - AV matmuls run with P^T as the stationary operand against [v | 1],
    so the softmax denominator falls out of column 128 of PSUM. av
    blocks sit at h*512 + tb2*129 in a 2-bank PSUM tile: one strided
    reciprocal + one broadcast tensor_tensor (stride-0 AP) normalizes a
    whole s-slice straight from PSUM fp32 into the bf16 output tile.
  - Queue split: adjacency + weights + x^T on the sync HWDGE ring
    (smalls issued first so the 2 MB bursts don't starve them); output
    stores per 2 s-slices via gpsimd SWDGE so they never block input
    prefetch. A dummy activation at t=0 pulls the ACT table load off the
    critical path. gpsimd is deliberately NOT used for elementwise work:
    its SBUF port lock against the (saturated) DVE costs more than it
    saves (measured +35 us).
  - Engine balance in steady state (~73 us): ACT = exp (59) + 3/4 of the
    weiT copies; DVE = mask-mult (37) + normalize (28) + 1/4 copies;
    PE = QK + AV (~60, not critical); DMA ~75 us union, overlapped.
"""

import numpy as np
import ml_dtypes

B, S, T, C, HS = 32, 8, 512, 128, 128
NCORES = 8
BPC = B // NCORES
TB = T // 128
UB = T // 128
SCALE = float(C) ** -0.5
SQ = 4  # s-slices per adjacency DMA / mult / exp block

_CACHED = None


def _build_module():
    import concourse.bacc as bacc
    import concourse.mybir as mybir
    from concourse import tile
    from concourse.ap import AP

    f32 = mybir.dt.float32
    f32r = mybir.dt.float32r
    bf16 = mybir.dt.bfloat16

    nc = bacc.Bacc("TRN2", target_bir_lowering=False, debug=False, num_devices=1)

    xT_d = nc.dram_tensor("xT", [BPC, C, T], bf16, kind="ExternalInput").ap()
    # adjacency pre-permuted on host to the exact SBUF layout:
    # [b, qi, p, s2, ub, t] so each partition reads one 16 KB run per DMA
    adjT_d = nc.dram_tensor(
        "adjT", [BPC, S // SQ, 128, SQ, UB, T], bf16, kind="ExternalInput"
    ).ap()
    wqk_d = nc.dram_tensor("Wqk", [C, C], bf16, kind="ExternalInput").ap()
    wv_d = nc.dram_tensor("Wv", [C, HS], bf16, kind="ExternalInput").ap()
    # [b, t, s, d] so each DMA line is >= 4*HS contiguous = 1 KB bf16
    out_d = nc.dram_tensor("out", [BPC, T, S, HS], bf16, kind="ExternalOutput").ap()

    with tile.TileContext(nc) as tc:
        with (
            tc.tile_pool(name="consts", bufs=1) as consts,
            tc.tile_pool(name="xp", bufs=BPC) as xp,
            tc.tile_pool(name="bpool", bufs=3) as bpool,
            tc.tile_pool(name="opool", bufs=3) as opool,
            tc.tile_pool(name="adjp", bufs=4) as adjp,
            tc.tile_pool(name="spool", bufs=4) as spool,
            tc.tile_pool(name="tiny", bufs=8) as tiny,
            tc.tile_pool(name="pav", bufs=3, space="PSUM") as pav,
            tc.tile_pool(name="psmall", bufs=2, space="PSUM") as psmall,
        ):
            # weights + x^T go FIRST on the sync ring so they land before
            # the 2 MB adjacency bursts start hogging the SDMA engines.
            # dummy activation: pulls the ACT exp-table load into the boot
            # window instead of the first real copy/exp on the critical path
            warm = consts.tile([128, 1], f32, tag="warm")
            nc.vector.memset(warm[:], 0.0)
            nc.scalar.activation(
                warm[:], warm[:], mybir.ActivationFunctionType.Exp
            )

            # PE warm-up in the boot shadow: ~3.4us of dummy matmuls flips
            # the HAM clock gate to 2.4 GHz before the first real matmul
            warmmat = consts.tile([128, T], bf16, tag="warmmat")
            nc.vector.memset(warmmat[:], 0.0)
            warm_ps = psmall.tile([128, T], f32, tag="ps")
            for _ in range(8):
                nc.tensor.matmul(warm_ps[:], warmmat[:, 0:128], warmmat[:])

            wqk_sb_t = consts.tile([C, C], bf16, tag="wqk")
            wv_sb_t = consts.tile([C, HS], bf16, tag="wv")
            nc.sync.dma_start(wqk_sb_t[:], wqk_d)
            nc.sync.dma_start(wv_sb_t[:], wv_d)
            wqk_sb, wv_sb = wqk_sb_t[:], wv_sb_t[:]

            xTs = []
            for b in range(BPC):
                xT_t = xp.tile([C, T], bf16, tag="xT")
                nc.sync.dma_start(xT_t[:], xT_d[b])
                xTs.append(xT_t[:])

            for b in range(BPC):
                xT = xTs[b]
                # ---- fused QK: g^T = (Wq Wk^T)^T-projected x, so that
                # wei^T = g x^T needs only one projection ----
                gT_ps = psmall.tile([C, T], f32, tag="ps")
                nc.tensor.matmul(gT_ps[:], wqk_sb, xT)
                gT = bpool.tile([C, T], bf16, tag="qT")
                nc.scalar.copy(gT[:], gT_ps[:])

                # ---- v natural [u, d] + ones column, bf16 ----
                vp = bpool.tile([128, UB, HS + 1], bf16, tag="vp")
                v_ps = psmall.tile([128, UB, HS], f32, tag="ps")
                for ub in range(UB):
                    nc.tensor.matmul(
                        v_ps[:, ub], xT[:, ub * 128 : (ub + 1) * 128], wv_sb
                    )
                nc.vector.tensor_copy(vp[:, :, 0:HS], v_ps[:])
                nc.vector.memset(vp[:, :, HS : HS + 1], 1.0)

                # ---- QK transposed: weiT [u, t] = (g x^T)^T = x g^T ----
                weiT = bpool.tile([128, UB, T], bf16, tag="weiT")
                for ub in range(UB):
                    weiT_ps = psmall.tile([128, T], f32, tag="ps")
                    nc.tensor.matmul(
                        weiT_ps[:], gT[:, ub * 128 : (ub + 1) * 128], xT
                    )
                    if ub < 3:
                        nc.scalar.copy(weiT[:, ub], weiT_ps[:])
                    else:
                        nc.vector.tensor_copy(weiT[:, ub], weiT_ps[:])

                for qi in range(S // SQ):
                    # 2 MB bf16 load of SQ adjacency slices (1 KB lines)
                    adj4 = adjp.tile([128, SQ, UB, T], bf16, tag="adj")
                    nc.sync.dma_start(adj4[:], adjT_d[b, qi])

                    outq = opool.tile([128, TB, SQ, HS], bf16, tag="outq")
                    for half in range(SQ // 2):
                        # prodT = adjT * weiT (bf16 2x, weiT bcast over s)
                        prod = spool.tile([128, 2, UB, T], bf16, tag="prod")
                        w_b = weiT[:].unsqueeze(1).broadcast_to((128, 2, UB, T))
                        nc.vector.tensor_tensor(
                            prod[:],
                            adj4[:, 2 * half : 2 * half + 2],
                            w_b,
                            mybir.AluOpType.mult,
                        )

                        # P^T = exp(scale * prodT)
                        pt = spool.tile([128, 2, UB, T], bf16, tag="pt")
                        nc.scalar.activation(
                            pt[:], prod[:], mybir.ActivationFunctionType.Exp,
                            scale=SCALE,
                        )

                        for s2 in range(2):
                            sq = 2 * half + s2
                            # av [128, 1024] spans 2 PSUM banks; block
                            # (h, tb2) at h*512 + tb2*129 so no matmul
                            # output crosses a bank.
                            av = pav.tile([128, 2 * 512], f32, tag="av")
                            for h in range(2):
                                for tb2 in range(2):
                                    tb = 2 * h + tb2
                                    off = h * 512 + tb2 * 129
                                    for ub in range(UB):
                                        nc.tensor.matmul(
                                            av[:, off : off + HS + 1],
                                            pt[
                                                :,
                                                s2,
                                                ub,
                                                tb * 128 : (tb + 1) * 128,
                                            ],
                                            vp[:, ub, :],
                                            start=(ub == 0),
                                            stop=(ub == UB - 1),
                                        )
                            # one strided reciprocal + one broadcast TT
                            av_ap = av[:]
                            pdim = list(av_ap.ap[0])
                            sums = AP(
                                av_ap.tensor,
                                av_ap.offset + HS,
                                [pdim, [512, 2], [129, 2], [1, 1]],
                            )
                            rcp = tiny.tile([128, 2, 2], f32, tag="rcp")
                            nc.vector.reciprocal(rcp[:], sums)
                            vals = AP(
                                av_ap.tensor,
                                av_ap.offset,
                                [pdim, [512, 2], [129, 2], [1, HS]],
                            )
                            r_b = rcp[:].unsqueeze(3).broadcast_to(
                                (128, 2, 2, HS)
                            )
                            o_ap = outq[:, :, sq, :].rearrange(
                                "p (h t2) d -> p h t2 d", h=2
                            )
                            nc.vector.tensor_tensor(
                                o_ap, vals, r_b, mybir.AluOpType.mult
                            )

                        nc.gpsimd.dma_start(
                            out_d[
                                b,
                                :,
                                SQ * qi + 2 * half : SQ * qi + 2 * half + 2,
                                :,
                            ].rearrange("(tb p) s d -> p tb s d", p=128),
                            outq[:, :, 2 * half : 2 * half + 2, :],
                        )

    nc.compile()
    return nc


def _get_module():
    global _CACHED
    if _CACHED is None:
        _CACHED = _build_module()
    return _CACHED


def run_on_hw(in_maps, trace=False, trace_kwargs=None):
    """Run the compiled module on the 8 NeuronCores. Returns BassKernelResults."""
    from concourse.bass_utils import run_bass_kernel_spmd
    from concourse.bass_interp import get_hw_module

    nc = _get_module()
    old_m = nc.m
    nc.m = get_hw_module(nc.m)
    try:
        return run_bass_kernel_spmd(
            nc,
            in_maps,
            core_ids=list(range(NCORES)),
            trace=trace,
            **(trace_kwargs or {}),
        )
    finally:
        nc.m = old_m


def make_in_maps(x, adjacent, Wq, Wk, Wv):
    x = np.ascontiguousarray(x, dtype=np.float32)
    Wqk = (
        np.asarray(Wk, dtype=np.float32) @ np.asarray(Wq, dtype=np.float32).T
    ).astype(ml_dtypes.bfloat16)  # [C, C]: weiT = x (Wk Wq^T) x^T
    Wv = np.ascontiguousarray(Wv, dtype=np.float32).astype(ml_dtypes.bfloat16)
    xT = np.ascontiguousarray(x.transpose(0, 2, 1)).astype(
        ml_dtypes.bfloat16
    )  # [B, C, T] bf16
    # [b, qi, p, s2, ub, t]: partition-contiguous runs for max DMA line size
    a6 = np.asarray(adjacent, dtype=np.float32).reshape(B, S // SQ, SQ, T, UB, 128)
    adjT = np.ascontiguousarray(a6.transpose(0, 1, 5, 2, 4, 3)).astype(
        ml_dtypes.bfloat16
    )
    return [
        {
            "xT": xT[c * BPC : (c + 1) * BPC],
            "adjT": adjT[c * BPC : (c + 1) * BPC],
            "Wqk": Wqk,
            "Wv": Wv,
        }
        for c in range(NCORES)
    ]


def kernel(**inputs) -> np.ndarray:
    in_maps = make_in_maps(
        inputs["x"], inputs["adjacent"], inputs["Wq"], inputs["Wk"], inputs["Wv"]
    )
    res = run_on_hw(in_maps)
    # per-core out: [BPC, T, S, HS] bf16 -> [BPC, S, T, HS] f32
    outs = [
        np.asarray(res.results[c]["out"])
        .astype(np.float32)
        .transpose(0, 2, 1, 3)
        for c in range(NCORES)
    ]
    return np.ascontiguousarray(np.concatenate(outs, axis=0))
